# revision 15
# baseline (speedup 1.0000x reference)
"""Multi-head self-attention (B=2, S=2048, D=1024, H=16, HD=64, causal) on 8 trn2 cores.

Sharding: core c = 4*b + g handles batch b and head group g (4 heads).
  - QKV projections are tensor-parallel over heads (column-split weights).
  - Output projection is row-split over the ctx dims; partial outputs are
    summed on the host (the "all-reduce"), bias added once.

Device kernel design (per core):
  - All matmuls use float32r (fast fp32 path, 1 cycle/row at N>=256).
  - Scores are computed TRANSPOSED: S^T[k, q] = K_h Q_h^T, so the exp output
    (P^T) is directly the moving operand of the AV matmul - no transposes.
  - Denominators come from a 64-wide ones block appended to V: the AV matmul
    replicates the softmax denominator across PSUM partitions 64-127, so the
    normalize is a plain reciprocal + elementwise multiply (no broadcasts).
  - exp without max-subtraction: |scores/8| <= ~3.1 for this input
    distribution, far inside the fp32 exp range.
  - Causal diagonal blocks are masked AFTER exp by gpsimd affine_select into
    separate ptd tiles; the AV matmul is split so each matmul operand has a
    single writer engine (the fp32r LDWEIGHTS op supports only ONE sync wait,
    so every matmul's dependencies must collapse to one semaphore).
  - Inputs are packed host-side into one [D, 2880] slab (xt|wq|wk|wv|ones) so
    each projection matmul waits on a single DMA semaphore.
"""

import sys

import numpy as np

if "/opt/trn_rl_repo" not in sys.path:
    sys.path.insert(0, "/opt/trn_rl_repo")

B, S, D, H, HD = 2, 2048, 1024, 16, 64
NH = 4          # heads per core
EL = NH * HD    # 256 local projection dims per core
P = 128
NT = S // P     # 16 n-tiles
DTI = D // P    # 8 d-tiles (contraction tiles for projections)
NCH = S // 512  # 4 q-chunks of 512
ET = EL // P    # 2 e-tiles of the local projection dims
VW = 2 * HD     # 128: V plus a 64-wide ones block (denominator replication)

OQ, OK_, OV, OO = S, S + EL, S + 2 * EL, S + 3 * EL
XW = S + 3 * EL + HD   # 2880 columns of the packed input slab


def group_sizes(n):
    """Split n k-tiles into exp groups of <=3 with the last group >= 2."""
    if n <= 3 and n != 1:
        return [n]
    if n % 3 == 1:
        return [3] * ((n - 4) // 3) + [2, 2]
    sizes = [3] * (n // 3)
    if n % 3:
        sizes.append(n % 3)
    return sizes


def build_bass():
    import concourse.bass as bass  # noqa: F401
    import concourse.mybir as mybir
    import concourse.tile as tile
    from concourse import bacc

    f32 = mybir.dt.float32
    f32r = mybir.dt.float32r
    EXP = mybir.ActivationFunctionType.Exp
    GE = mybir.AluOpType.is_ge
    MUL = mybir.AluOpType.mult

    nc = bacc.Bacc("TRN2", target_bir_lowering=False, debug=False, num_devices=8)

    xw_d = nc.dram_tensor("xw", [D, XW], f32r, kind="ExternalInput").ap()
    wot_d = nc.dram_tensor("wot", [EL, D], f32r, kind="ExternalInput").ap()
    out_d = nc.dram_tensor("out", [S, D], f32, kind="ExternalOutput").ap()

    with tile.TileContext(nc) as tc:
        with tc.tile_pool(name="persist", bufs=1) as persist:
            qt = [persist.tile([P, S], f32r, tag=f"qt{e}", name=f"qt{e}")
                  for e in range(ET)]
            kt = [persist.tile([P, S], f32r, tag=f"kt{e}", name=f"kt{e}")
                  for e in range(ET)]
            vaug = [persist.tile([P, NH, VW], f32r, tag=f"va{n}", name=f"va{n}")
                    for n in range(NT)]
            ctxn = [persist.tile([P, S], f32r, tag=f"cx{e}", name=f"cx{e}")
                    for e in range(ET)]
            wot_sb = [persist.tile([P, D], f32r, tag=f"wo{e}", name=f"wo{e}")
                      for e in range(ET)]

            # ---------------- Phase A: load + QKV projections ----------------
            with (
                tc.tile_pool(name="xw", bufs=1) as xw,
                tc.tile_pool(name="pa", bufs=1, space="PSUM") as pa,
            ):
                xw_sb = []
                for dt_ in range(DTI):
                    t = xw.tile([P, XW], f32r, tag=f"xw{dt_}", name=f"xw{dt_}")
                    nc.sync.dma_start(t[:], xw_d[P * dt_:P * dt_ + P, :])
                    xw_sb.append(t)
                # wot staged then re-written through DVE so phase-C matmuls
                # wait on a single (DVE) semaphore.
                for e in range(ET):
                    stage = xw.tile([P, D], f32r, tag=f"ws{e}", name=f"ws{e}")
                    nc.sync.dma_start(stage[:], wot_d[P * e:P * e + P, :])
                    nc.vector.tensor_copy(wot_sb[e][:], stage[:])

                # V projection first (so vaug's ACT ticks precede qt/kt's):
                # natural layout [n, e], scattered into vaug + ones block.
                for rnd in range(2):
                    psv = {
                        i: pa.tile([P, 256], f32, tag=f"pa{i}", name=f"pv{rnd}_{i}")
                        for i in range(8)
                    }
                    for dt_ in range(DTI):
                        for i in range(8):
                            n = 8 * rnd + i
                            nc.tensor.matmul(
                                psv[i][:],
                                lhsT=xw_sb[dt_][:, P * n:P * n + P],
                                rhs=xw_sb[dt_][:, OV:OV + EL],
                                start=(dt_ == 0),
                                stop=(dt_ == DTI - 1),
                            )
                    for i in range(8):
                        n = 8 * rnd + i
                        src = psv[i][:].rearrange("p (h w) -> p h w", h=NH)
                        nc.scalar.copy(vaug[n][:, :, 0:HD], src)
                        for hh in range(NH):
                            nc.scalar.copy(
                                vaug[n][:, hh, HD:VW],
                                xw_sb[0][:, OO:OO + HD],
                            )

                # Q and K projections: out^T layout [e, n]
                for off, dst in ((OQ, qt), (OK_, kt)):
                    ps = {
                        (e, c): pa.tile(
                            [P, 512], f32,
                            tag=f"pa{NCH * e + c}", name=f"pq{off}_{e}{c}",
                        )
                        for e in range(ET)
                        for c in range(NCH)
                    }
                    for dt_ in range(DTI):
                        for e in range(ET):
                            for c in range(NCH):
                                nc.tensor.matmul(
                                    ps[(e, c)][:],
                                    lhsT=xw_sb[dt_][:, off + P * e:off + P * e + P],
                                    rhs=xw_sb[dt_][:, 512 * c:512 * c + 512],
                                    start=(dt_ == 0),
                                    stop=(dt_ == DTI - 1),
                                )
                    for e in range(ET):
                        for c in range(NCH):
                            nc.scalar.copy(
                                dst[e][:, 512 * c:512 * c + 512], ps[(e, c)][:]
                            )

            # ---------------- Phase B: attention ----------------
            with (
                tc.tile_pool(name="ptp", bufs=2) as ptp,
                tc.tile_pool(name="aux", bufs=1) as aux,
                tc.tile_pool(name="psb", bufs=1, space="PSUM") as psb,
            ):
                def emit_st(c, h):
                    """scores^T + exp for head h, q-chunk c -> (pt, ptd[j])."""
                    e, off = h // 2, HD * (h % 2)
                    nkt = 4 * c + 4
                    pt = ptp.tile([P, S * NCH], f32r, tag="pt", name="pt")
                    ptd = [
                        ptp.tile([P, P], f32r, tag=f"ptd{j}", name=f"ptd{j}",
                                 bufs=2)
                        for j in range(NH)
                    ]
                    g0 = 0
                    for gs in group_sizes(nkt):
                        sp = psb.tile([P, 512 * 3], f32, tag="sp", bufs=2,
                                      name="sp")
                        for j in range(gs):
                            kti = g0 + j
                            nc.tensor.matmul(
                                sp[:, 512 * j:512 * j + 512],
                                lhsT=kt[e][off:off + HD, P * kti:P * kti + P],
                                rhs=qt[e][off:off + HD, 512 * c:512 * c + 512],
                                start=True,
                                stop=True,
                            )
                        nc.scalar.activation(
                            pt[:, 512 * g0:512 * (g0 + gs)],
                            sp[:, 0:512 * gs],
                            EXP,
                            scale=0.125,
                        )
                        # masked diagonal tiles for the k-tiles in this group
                        for j in range(gs):
                            kti = g0 + j
                            dj = kti - 4 * c
                            if dj >= 0:
                                o = P * dj
                                nc.gpsimd.affine_select(
                                    out=ptd[dj][:],
                                    in_=pt[:, 512 * kti + o:512 * kti + o + P],
                                    pattern=[[1, P]],
                                    compare_op=GE,
                                    fill=0.0,
                                    base=0,
                                    channel_multiplier=-1,
                                )
                        g0 += gs
                    return pt, ptd

                def emit_av(c, h, pt, ptd):
                    e, doff = h // 2, HD * (h % 2)
                    nkt = 4 * c + 4
                    ctx = psb.tile([P, 512], f32, tag="ctx", bufs=2, name="ctx")
                    first = True
                    for kti in range(nkt):
                        dj = kti - 4 * c
                        if dj < 0:
                            nc.tensor.matmul(
                                ctx[:],
                                lhsT=vaug[kti][:, h, :],
                                rhs=pt[:, 512 * kti:512 * kti + 512],
                                start=first,
                                stop=False,
                            )
                            first = False
                        else:
                            o = P * dj
                            if o + P < 512:
                                nc.tensor.matmul(
                                    ctx[:, o + P:512],
                                    lhsT=vaug[kti][:, h, :],
                                    rhs=pt[:, 512 * kti + o + P:512 * kti + 512],
                                    start=first,
                                    stop=False,
                                )
                                first = False
                            nc.tensor.matmul(
                                ctx[:, o:o + P],
                                lhsT=vaug[kti][:, h, :],
                                rhs=ptd[dj][:],
                                start=False,
                                stop=(kti == nkt - 1),
                            )
                    # free the PSUM bank via one ACT copy, then normalize:
                    # rows 64-127 hold the denominator (ones-block product).
                    cu = aux.tile([P, 512], f32, tag="cu", bufs=3, name="cu")
                    nc.scalar.copy(cu[:], ctx[:])
                    recip = aux.tile([HD, 512], f32, tag="recip", bufs=2,
                                     name="recip")
                    nc.vector.reciprocal(recip[:], cu[HD:P, :])
                    nc.vector.scalar_tensor_tensor(
                        out=ctxn[e][doff:doff + HD, 512 * c:512 * c + 512],
                        in0=cu[0:HD, :],
                        scalar=1.0,
                        in1=recip[:],
                        op0=MUL,
                        op1=MUL,
                    )

                for c in range(NCH):
                    work = {0: emit_st(c, 0)}
                    for h in range(NH):
                        if h + 1 < NH:
                            work[h + 1] = emit_st(c, h + 1)
                        pt, ptd = work.pop(h)
                        emit_av(c, h, pt, ptd)

            # ---------------- Phase C: output projection ----------------
            with (
                tc.tile_pool(name="osb", bufs=4) as osb,
                tc.tile_pool(name="pc", bufs=4, space="PSUM") as pc,
            ):
                for n in range(NT):
                    for ec in range(2):
                        ps = pc.tile([P, 512], f32, tag="pc", name="pc")
                        for e in range(ET):
                            nc.tensor.matmul(
                                ps[:],
                                lhsT=ctxn[e][:, P * n:P * n + P],
                                rhs=wot_sb[e][:, 512 * ec:512 * ec + 512],
                                start=(e == 0),
                                stop=(e == ET - 1),
                            )
                        ot = osb.tile([P, 512], f32, tag="ot", name="ot")
                        nc.vector.tensor_copy(ot[:], ps[:])
                        nc.sync.dma_start(
                            out_d[P * n:P * n + P, 512 * ec:512 * ec + 512],
                            ot[:],
                        )

    nc.finalize()
    return nc


def shard_inputs(x, Wq, Wk, Wv, Wo):
    """Build the per-core input maps (host-side resharding)."""
    in_maps = []
    ones = np.ones((D, HD), np.float32)
    for core in range(8):
        b, g = core // 4, core % 4
        sl = slice(EL * g, EL * g + EL)
        xw = np.concatenate(
            [
                x[b].T.astype(np.float32),
                Wq[sl, :].T.astype(np.float32),
                Wk[sl, :].T.astype(np.float32),
                Wv[sl, :].T.astype(np.float32),
                ones,
            ],
            axis=1,
        )
        in_maps.append(
            {
                "xw": np.ascontiguousarray(xw),
                "wot": np.ascontiguousarray(Wo[:, sl].T.astype(np.float32)),
            }
        )
    return in_maps


_CACHE = {}


def kernel(x, Wq, Wk, Wv, Wo, bo, _want_results=False, _trace=False):
    from concourse import bass_utils

    x = np.asarray(x)
    Wq, Wk, Wv, Wo, bo = (np.asarray(a) for a in (Wq, Wk, Wv, Wo, bo))

    if "nc" not in _CACHE:
        _CACHE["nc"] = build_bass()
    nc = _CACHE["nc"]

    in_maps = shard_inputs(x, Wq, Wk, Wv, Wo)
    res = bass_utils.run_bass_kernel_spmd(
        nc, in_maps, core_ids=list(range(8)), trace=_trace
    )

    out = np.zeros((B, S, D), np.float32)
    for core in range(8):
        out[core // 4] += res.results[core]["out"]
    out += bo.astype(np.float32)
    if _want_results:
        return out, res
    return out


# revision 17
# speedup vs baseline: 1.2351x; 1.2351x over previous
"""Multi-head self-attention (B=2, S=2048, D=1024, H=16, HD=64, causal) on 8 trn2 cores.

Sharding: core c = 4*b + g handles batch b and head group g (4 heads).
  - QKV projections are tensor-parallel over heads (column-split weights).
  - Output projection is row-split over the ctx dims; partial outputs are
    summed on the host (the "all-reduce"), bias added once.

Device kernel design (per core):
  - Scores are computed TRANSPOSED: S^T[k, q] = K_h Q_h^T, so the exp output
    (P^T) is directly the moving operand of the AV matmul - no transposes.
  - Denominators come from a 64-wide ones block appended to V: the AV matmul
    replicates the softmax denominator across PSUM partitions 64-127, so the
    normalize is a plain reciprocal + elementwise multiply (no broadcasts).
  - exp without max-subtraction: |scores/8| <= ~3.1 for this input
    distribution, far inside the fp32 exp range.
  - Causal diagonal blocks are zeroed after exp by gpsimd affine_select.
  - Output projection is interleaved per q-chunk so phase C overlaps phase B.
  - Matmul dtype is bf16 by default (fp32 PSUM accumulation): the fp32r path
    runs at 2 cycles/row and its fp32_mode=HIGH matmuls do not register as
    PE activity for the HAM clock gate, pinning the PE at 1.2 GHz.
"""

import sys

import numpy as np

if "/opt/trn_rl_repo" not in sys.path:
    sys.path.insert(0, "/opt/trn_rl_repo")

B, S, D, H, HD = 2, 2048, 1024, 16, 64
NH = 4          # heads per core
EL = NH * HD    # 256 local projection dims per core
P = 128
NT = S // P     # 16 n-tiles
DTI = D // P    # 8 d-tiles (contraction tiles for projections)
NCH = S // 512  # 4 q-chunks of 512
ET = EL // P    # 2 e-tiles of the local projection dims
VW = 2 * HD     # 128: V plus a 64-wide ones block (denominator replication)

OQ, OK_, OV, OO = S, S + EL, S + 2 * EL, S + 3 * EL
XW = S + 3 * EL + HD   # 2880 columns of the packed input slab

MM_DTYPE = "bfloat16"   # matmul operand dtype: "bfloat16" or "float32r"


def build_bass(mm_dtype=MM_DTYPE):
    import concourse.bass as bass  # noqa: F401
    import concourse.mybir as mybir
    import concourse.tile as tile
    from concourse import bacc

    f32 = mybir.dt.float32
    mdt = getattr(mybir.dt, mm_dtype)
    EXP = mybir.ActivationFunctionType.Exp
    GE = mybir.AluOpType.is_ge
    MUL = mybir.AluOpType.mult

    nc = bacc.Bacc("TRN2", target_bir_lowering=False, debug=False, num_devices=8)

    xw_d = nc.dram_tensor("xw", [D, XW], mdt, kind="ExternalInput").ap()
    wot_d = nc.dram_tensor("wot", [EL, D], mdt, kind="ExternalInput").ap()
    out_d = nc.dram_tensor("out", [S, D], f32, kind="ExternalOutput").ap()

    with tile.TileContext(nc) as tc:
        with tc.tile_pool(name="persist", bufs=1) as persist:
            qt = [persist.tile([P, S], mdt, tag=f"qt{e}", name=f"qt{e}")
                  for e in range(ET)]
            kt = [persist.tile([P, S], mdt, tag=f"kt{e}", name=f"kt{e}")
                  for e in range(ET)]
            vaug = [persist.tile([P, NH, VW], mdt, tag=f"va{n}", name=f"va{n}")
                    for n in range(NT)]
            ctxn = [persist.tile([P, S], mdt, tag=f"cx{e}", name=f"cx{e}")
                    for e in range(ET)]
            wot_sb = [persist.tile([P, D], mdt, tag=f"wo{e}", name=f"wo{e}")
                      for e in range(ET)]

            # ---------------- Phase A: load + QKV projections ----------------
            with (
                tc.tile_pool(name="xw", bufs=1) as xw,
                tc.tile_pool(name="pa", bufs=1, space="PSUM") as pa,
            ):
                xw_sb = []
                for dt_ in range(DTI):
                    t = xw.tile([P, XW], mdt, tag=f"xw{dt_}", name=f"xw{dt_}")
                    nc.sync.dma_start(t[:], xw_d[P * dt_:P * dt_ + P, :])
                    xw_sb.append(t)
                for e in range(ET):
                    nc.sync.dma_start(wot_sb[e][:], wot_d[P * e:P * e + P, :])
                # ones blocks of vaug straight from the slab's ones columns
                for n in range(NT):
                    src = bass.AP(
                        tensor=xw_d.tensor,
                        offset=OO,
                        ap=[[XW, P], [0, NH], [1, HD]],
                    )
                    nc.sync.dma_start(vaug[n][:, :, HD:VW], src)

                # V projection first: natural layout [n, e] into vaug
                for rnd in range(2):
                    psv = {
                        i: pa.tile([P, 256], f32, tag=f"pa{i}", name=f"pv{rnd}_{i}")
                        for i in range(8)
                    }
                    for dt_ in range(DTI):
                        for i in range(8):
                            n = 8 * rnd + i
                            nc.tensor.matmul(
                                psv[i][:],
                                lhsT=xw_sb[dt_][:, P * n:P * n + P],
                                rhs=xw_sb[dt_][:, OV:OV + EL],
                                start=(dt_ == 0),
                                stop=(dt_ == DTI - 1),
                            )
                    for i in range(8):
                        n = 8 * rnd + i
                        src = psv[i][:].rearrange("p (h w) -> p h w", h=NH)
                        nc.scalar.copy(vaug[n][:, :, 0:HD], src)

                # Q and K projections: out^T layout [e, n]
                for off, dst in ((OQ, qt), (OK_, kt)):
                    ps = {
                        (e, c): pa.tile(
                            [P, 512], f32,
                            tag=f"pa{NCH * e + c}", name=f"pq{off}_{e}{c}",
                        )
                        for e in range(ET)
                        for c in range(NCH)
                    }
                    for dt_ in range(DTI):
                        for e in range(ET):
                            for c in range(NCH):
                                nc.tensor.matmul(
                                    ps[(e, c)][:],
                                    lhsT=xw_sb[dt_][:, off + P * e:off + P * e + P],
                                    rhs=xw_sb[dt_][:, 512 * c:512 * c + 512],
                                    start=(dt_ == 0),
                                    stop=(dt_ == DTI - 1),
                                )
                    for e in range(ET):
                        for c in range(NCH):
                            nc.scalar.copy(
                                dst[e][:, 512 * c:512 * c + 512], ps[(e, c)][:]
                            )

            # ---------- Phases B+C: attention + interleaved out-proj ----------
            with (
                tc.tile_pool(name="ptp", bufs=3) as ptp,
                tc.tile_pool(name="aux", bufs=1) as aux,
                tc.tile_pool(name="osb", bufs=4) as osb,
                tc.tile_pool(name="psb", bufs=1, space="PSUM") as psb,
            ):
                GK = 2  # k-tiles per exp group

                def emit_st(c, h):
                    """scores^T + exp + diag masks for head h, q-chunk c."""
                    e, off = h // 2, HD * (h % 2)
                    nkt = 4 * c + 4
                    pt = ptp.tile([P, S * NCH], mdt, tag="pt", name="pt")
                    for g0 in range(0, nkt, GK):
                        sp = psb.tile([P, 512 * GK], f32, tag="sp", bufs=2,
                                      name="sp")
                        for j in range(GK):
                            kti = g0 + j
                            nc.tensor.matmul(
                                sp[:, 512 * j:512 * j + 512],
                                lhsT=kt[e][off:off + HD, P * kti:P * kti + P],
                                rhs=qt[e][off:off + HD, 512 * c:512 * c + 512],
                                start=True,
                                stop=True,
                            )
                        nc.scalar.activation(
                            pt[:, 512 * g0:512 * (g0 + GK)],
                            sp[:, 0:512 * GK],
                            EXP,
                            scale=0.125,
                        )
                        # zero invalid (q < k) entries of diagonal blocks
                        for j in range(GK):
                            kti = g0 + j
                            dj = kti - 4 * c
                            if dj >= 0:
                                o = P * dj
                                blk = pt[:, 512 * kti + o:512 * kti + o + P]
                                nc.gpsimd.affine_select(
                                    out=blk,
                                    in_=blk,
                                    pattern=[[1, P]],
                                    compare_op=GE,
                                    fill=0.0,
                                    base=0,
                                    channel_multiplier=-1,
                                )
                    return pt

                def emit_av(c, h, pt):
                    e, doff = h // 2, HD * (h % 2)
                    nkt = 4 * c + 4
                    ctx = psb.tile([P, 512], f32, tag="ctx", bufs=2, name="ctx")
                    for kti in range(nkt):
                        o = max(0, P * (kti - 4 * c))
                        nc.tensor.matmul(
                            ctx[:, o:512],
                            lhsT=vaug[kti][:, h, :],
                            rhs=pt[:, 512 * kti + o:512 * kti + 512],
                            start=(kti == 0),
                            stop=(kti == nkt - 1),
                        )
                    # rows 64-127 hold the denominator; copy out, reciprocal,
                    # multiply. cu copy on DVE keeps ACT exp-only in phase B.
                    cu = aux.tile([P, 512], f32, tag="cu", bufs=3, name="cu")
                    nc.vector.tensor_copy(cu[:], ctx[:])
                    recip = aux.tile([HD, 512], f32, tag="recip", bufs=2,
                                     name="recip")
                    nc.vector.reciprocal(recip[:], cu[HD:P, :])
                    nc.vector.scalar_tensor_tensor(
                        out=ctxn[e][doff:doff + HD, 512 * c:512 * c + 512],
                        in0=cu[0:HD, :],
                        scalar=1.0,
                        in1=recip[:],
                        op0=MUL,
                        op1=MUL,
                    )

                def emit_outproj(c):
                    for nt_ in range(4 * c, 4 * c + 4):
                        for ec in range(2):
                            ps = psb.tile([P, 512], f32, tag="pc", bufs=2,
                                          name="pc")
                            for e in range(ET):
                                nc.tensor.matmul(
                                    ps[:],
                                    lhsT=ctxn[e][:, P * nt_:P * nt_ + P],
                                    rhs=wot_sb[e][:, 512 * ec:512 * ec + 512],
                                    start=(e == 0),
                                    stop=(e == ET - 1),
                                )
                            ot = osb.tile([P, 512], f32, tag="ot", name="ot")
                            nc.vector.tensor_copy(ot[:], ps[:])
                            nc.sync.dma_start(
                                out_d[P * nt_:P * nt_ + P,
                                      512 * ec:512 * ec + 512],
                                ot[:],
                            )

                work = {(0, 0): emit_st(0, 0)}

                def st_ahead(c, h):
                    # emit the next head's scores one step ahead for pipelining
                    if h + 1 < NH:
                        work[(c, h + 1)] = emit_st(c, h + 1)
                    elif c + 1 < NCH:
                        work[(c + 1, 0)] = emit_st(c + 1, 0)

                for c in range(NCH):
                    for h in range(NH):
                        st_ahead(c, h)
                        emit_av(c, h, work.pop((c, h)))
                    emit_outproj(c)

    nc.finalize()
    return nc


def shard_inputs(x, Wq, Wk, Wv, Wo, np_dtype):
    """Build the per-core input maps (host-side resharding)."""
    in_maps = []
    ones = np.ones((D, HD), np.float32)
    for core in range(8):
        b, g = core // 4, core % 4
        sl = slice(EL * g, EL * g + EL)
        xw = np.concatenate(
            [
                x[b].T.astype(np.float32),
                Wq[sl, :].T.astype(np.float32),
                Wk[sl, :].T.astype(np.float32),
                Wv[sl, :].T.astype(np.float32),
                ones,
            ],
            axis=1,
        )
        in_maps.append(
            {
                "xw": np.ascontiguousarray(xw.astype(np_dtype)),
                "wot": np.ascontiguousarray(
                    Wo[:, sl].T.astype(np.float32).astype(np_dtype)
                ),
            }
        )
    return in_maps


_CACHE = {}


def kernel(x, Wq, Wk, Wv, Wo, bo, _want_results=False, _trace=False,
           _mm_dtype=MM_DTYPE):
    import concourse.mybir as mybir
    from concourse import bass_utils

    x = np.asarray(x)
    Wq, Wk, Wv, Wo, bo = (np.asarray(a) for a in (Wq, Wk, Wv, Wo, bo))

    key = ("nc", _mm_dtype)
    if key not in _CACHE:
        _CACHE[key] = build_bass(_mm_dtype)
    nc = _CACHE[key]

    np_dtype = mybir.dt.np(getattr(mybir.dt, _mm_dtype))
    in_maps = shard_inputs(x, Wq, Wk, Wv, Wo, np_dtype)
    res = bass_utils.run_bass_kernel_spmd(
        nc, in_maps, core_ids=list(range(8)), trace=_trace
    )

    out = np.zeros((B, S, D), np.float32)
    for core in range(8):
        out[core // 4] += res.results[core]["out"]
    out += bo.astype(np.float32)
    if _want_results:
        return out, res
    return out
